# revision 5
# baseline (speedup 1.0000x reference)
"""DecoderRNN Trainium2 Bass kernel.

Data-parallel over batch: 8 cores x 32 batch rows each. Each core runs the
full T=128 recurrence for its shard; weights are replicated. The large
input projection (x @ W_x.T, 85% of FLOPs) is precomputed for all steps as
one big tiled matmul into DRAM; the recurrent loop then only does the
state/embedding projections per step.

Loop layout is feature-transposed ([feature-partitions, batch-free]) so the
elementwise gate math uses all 128 partitions. The state matmul uses a
two-stage trick: stage 1 packs the 8 K-chunks into the PE array 4-at-a-time
via tile_position col-groups (stationary = state slices, moving = W rows),
stage 2 reduces the 4 partition groups AND transposes via a matmul with a
block-ones matrix.

Precision: px/ps/pe matmuls in fp16 (inputs host-cast / device-rounded),
accumulation fp32; pred/argmax/embedding path exactly fp32.
"""

import numpy as np
import ml_dtypes

import concourse.bass as bass
import concourse.bacc as bacc
import concourse.mybir as mybir
from concourse.tile import TileContext
from concourse.bass_utils import run_bass_kernel_spmd

F32 = mybir.dt.float32
F16 = mybir.dt.float16
I32 = mybir.dt.int32
U32 = mybir.dt.uint32
AF = mybir.ActivationFunctionType
OP = mybir.AluOpType

T, B = 128, 256
D_IN, E, H, C = 2048, 100, 1024, 151
D = D_IN + E
BL = B // 8            # batch per core
G6, G5 = 6 * H, 5 * H
KD = D_IN // 128       # 16 contraction chunks for px
NTOK = T * BL          # 4096 tokens per core
P = 128


def build_decoder(t_steps=T, px_blocks=None, px_fp32=False):
    """Build the Bass kernel. t_steps/px_blocks < full for cheap sim tests."""
    xdt = F32 if px_fp32 else F16
    n_halves = 2 if px_fp32 else 1
    xcols = NTOK // n_halves
    blocks_per_half = xcols // 512
    if px_blocks is None:
        px_blocks = NTOK // 512

    nc = bacc.Bacc("TRN2", target_bir_lowering=False, num_devices=8)

    # ---- DRAM I/O ----
    xT_d = nc.dram_tensor("xT", [D_IN, NTOK], xdt, kind="ExternalInput")
    WxT_d = nc.dram_tensor("WxT", [D_IN, G6], xdt, kind="ExternalInput")
    WsT_d = nc.dram_tensor("WsT", [H, G5], F16, kind="ExternalInput")
    WeT_d = nc.dram_tensor("WeT", [P, G6], F16, kind="ExternalInput")
    WoT_d = nc.dram_tensor("WoT", [H, C], F32, kind="ExternalInput")
    obj_d = nc.dram_tensor("objp", [2 * P, E], F32, kind="ExternalInput")
    bias_d = nc.dram_tensor("biasf", [48, P], F32, kind="ExternalInput")
    bout_d = nc.dram_tensor("boutr", [BL, C], F32, kind="ExternalInput")
    iota_d = nc.dram_tensor("iotar", [BL, 256], F32, kind="ExternalInput")
    ones_d = nc.dram_tensor("ones4", [P, BL], F32, kind="ExternalInput")
    id_d = nc.dram_tensor("ident", [P, P], F32, kind="ExternalInput")
    lab_d = nc.dram_tensor("labT", [BL, T], I32, kind="ExternalInput")
    emb0_d = nc.dram_tensor("emb0T", [P, BL], F16, kind="ExternalInput")

    dists_d = nc.dram_tensor("dists", [T, BL, C], F32, kind="ExternalOutput")
    comms_d = nc.dram_tensor("comms", [BL, T], I32, kind="ExternalOutput")

    with TileContext(nc) as tc:
        with tc.tile_pool(name="dram", bufs=1, space="DRAM") as dpool, \
             tc.tile_pool(name="wsmall", bufs=1) as wp:
            # small persistent weights
            WeT = wp.tile([P, G6], F16)
            nc.sync.dma_start(WeT[:], WeT_d[:])
            WoT = wp.tile([P, 8, C], F32)
            nc.sync.dma_start(WoT[:], WoT_d[:].rearrange("(k p) c -> p k c", p=P))
            obj = wp.tile([P, 2, E], F32)
            nc.sync.dma_start(obj[:], obj_d[:].rearrange("(k p) e -> p k e", p=P))
            bias = wp.tile([P, 48], F32)
            nc.sync.dma_start(bias[:], bias_d[:].rearrange("m p -> p m"))
            bout = wp.tile([BL, C], F32)
            nc.sync.dma_start(bout[:], bout_d[:])
            iota = wp.tile([BL, 256], F32)
            nc.sync.dma_start(iota[:], iota_d[:])
            ones4 = wp.tile([P, BL], F32)
            nc.sync.dma_start(ones4[:], ones_d[:])
            ident = wp.tile([P, P], F32)
            nc.sync.dma_start(ident[:], id_d[:])
            lab = wp.tile([BL, T], I32)
            nc.sync.dma_start(lab[:], lab_d[:])
            emb0 = wp.tile([P, BL], F16)
            nc.sync.dma_start(emb0[:], emb0_d[:])

            px_dram = dpool.tile([48, NTOK // 512, P, 512], F32)

            # ================= PHASE A: px precompute =================
            with tc.tile_pool(name="pxc", bufs=1) as xp, \
                 tc.tile_pool(name="pxw", bufs=2) as wxp, \
                 tc.tile_pool(name="pxcp", bufs=4) as cpp, \
                 tc.tile_pool(name="pxps", bufs=4, space="PSUM") as pps:
                for h in range(n_halves):
                    xsb = xp.tile([P, KD, xcols], xdt, tag="xsb")
                    nc.sync.dma_start(
                        xsb[:],
                        xT_d[:, h * xcols:(h + 1) * xcols].rearrange(
                            "(k p) n -> p k n", p=P))
                    for m in range(48):
                        wsb = wxp.tile([P, KD, P], xdt, tag="wx")
                        nc.sync.dma_start(
                            wsb[:],
                            WxT_d[:, m * P:(m + 1) * P].rearrange(
                                "(k p) f -> p k f", p=P))
                        for nb in range(blocks_per_half):
                            gb = h * blocks_per_half + nb
                            if gb >= px_blocks:
                                continue
                            ps = pps.tile([P, 512], F32, tag="ps")
                            for k in range(KD):
                                nc.tensor.matmul(
                                    ps[:], wsb[:, k, :],
                                    xsb[:, k, nb * 512:(nb + 1) * 512],
                                    start=(k == 0), stop=(k == KD - 1))
                            cp = cpp.tile([P, 512], F32, tag="cp")
                            # copy + fold the (b_in + b_state) bias in
                            nc.scalar.activation(cp[:], ps[:], AF.Identity,
                                                 bias=bias[:, m:m + 1], scale=1.0)
                            nc.sync.dma_start(px_dram[m, gb], cp[:])

            # ================= PHASE B: recurrence =================
            with tc.tile_pool(name="wbig", bufs=1) as wb, \
                 tc.tile_pool(name="pxin", bufs=3) as pxp, \
                 tc.tile_pool(name="cpl", bufs=4) as cpl, \
                 tc.tile_pool(name="gat", bufs=2) as gp, \
                 tc.tile_pool(name="sta", bufs=2) as sp, \
                 tc.tile_pool(name="arg", bufs=2) as ap, \
                 tc.tile_pool(name="ps1", bufs=2, space="PSUM") as ps1p, \
                 tc.tile_pool(name="ps2", bufs=1, space="PSUM") as ps2p, \
                 tc.tile_pool(name="psm", bufs=1, space="PSUM") as psm:

                WsT = wb.tile([P, 8, G5], F16)
                nc.sync.dma_start(WsT[:], WsT_d[:].rearrange("(k p) g -> p k g", p=P))
                comms_sb = wb.tile([BL, T], I32)

                prev_state = wb.tile([P, 8, BL], F16)
                nc.vector.memset(prev_state[:], 0)
                prev_mem = wb.tile([P, 8, BL], F32)
                nc.vector.memset(prev_mem[:], 0)
                prev_emb = emb0

                FUNC = [AF.Sigmoid, AF.Sigmoid, AF.Tanh, AF.Sigmoid, AF.Sigmoid]

                for t in range(t_steps):
                    # px for this step: [128, 48, 32]
                    pxt = pxp.tile([P, 48, BL], F32, tag="px")
                    nc.sync.dma_start(
                        pxt[:],
                        px_dram[:, t // 16, :,
                                (t % 16) * BL:(t % 16 + 1) * BL].rearrange(
                                    "m p j -> p m j"))

                    # stage 1: psum1[32g+n, f] = partial ps sums (+ pe in grp0)
                    cps = []
                    for c in range(12):
                        ps1 = ps1p.tile([P, 512], F32, tag="s1")
                        csl = slice(c * 512, (c + 1) * 512)
                        if c < 10:
                            for r in range(2):
                                for g in range(4):
                                    k = r * 4 + g
                                    nc.tensor.matmul(
                                        ps1[32 * g:32 * (g + 1), :],
                                        prev_state[:, k, :], WsT[:, k, csl],
                                        start=(r == 0), stop=(r == 1 and g == 3),
                                        tile_position=(0, 32 * g),
                                        skip_group_check=True)
                            nc.tensor.matmul(
                                ps1[0:32, :], prev_emb[0:E, :], WeT[0:E, csl],
                                start=False, stop=True, tile_position=(0, 0),
                                skip_group_check=True)
                        else:
                            # bypass cols: pe only, replicated in all 4 groups
                            # (WeT bypass cols pre-scaled by 0.25 on host)
                            for g in range(4):
                                nc.tensor.matmul(
                                    ps1[32 * g:32 * (g + 1), :],
                                    prev_emb[0:E, :], WeT[0:E, csl],
                                    start=True, stop=True,
                                    tile_position=(0, 32 * g),
                                    skip_group_check=True)
                        cp = cpl.tile([P, 512], F32, tag="cpl")
                        if c % 2 == 0:
                            nc.scalar.copy(cp[:], ps1[:])
                        else:
                            nc.vector.tensor_copy(cp[:], ps1[:])
                        cps.append(cp)

                    # stage 2: reduce groups + transpose -> ps2[p, fc, b]
                    ps2 = ps2p.tile([P, 48, BL], F32, tag="s2")
                    for c in range(12):
                        for s in range(4):
                            fc = c * 4 + s
                            nc.tensor.matmul(
                                ps2[:, fc, :], cps[c][:, s * P:(s + 1) * P],
                                ones4[:], start=True, stop=True)

                    # gates
                    gates = []
                    for g in range(5):
                        pre = gp.tile([P, 8, BL], F32, tag=f"pre{g}")
                        nc.vector.tensor_add(pre[:], ps2[:, 8 * g:8 * (g + 1), :],
                                             pxt[:, 8 * g:8 * (g + 1), :])
                        gt = gp.tile([P, 8, BL], F32, tag=f"g{g}")
                        nc.scalar.activation(gt[:], pre[:], FUNC[g])
                        gates.append(gt)
                    ig, fg, mi, og, hg = gates
                    pb = gp.tile([P, 8, BL], F32, tag="pb")
                    nc.vector.tensor_add(pb[:], ps2[:, 40:48, :], pxt[:, 40:48, :])

                    t1 = gp.tile([P, 8, BL], F32, tag="t1")
                    nc.vector.tensor_mul(t1[:], ig[:], mi[:])
                    t2 = gp.tile([P, 8, BL], F32, tag="t2")
                    nc.vector.tensor_mul(t2[:], fg[:], prev_mem[:])
                    mem_new = sp.tile([P, 8, BL], F32, tag="mem")
                    nc.vector.tensor_add(mem_new[:], t1[:], t2[:])
                    tnh = gp.tile([P, 8, BL], F32, tag="tnh")
                    nc.scalar.activation(tnh[:], mem_new[:], AF.Tanh)
                    out1 = gp.tile([P, 8, BL], F32, tag="out1")
                    nc.vector.tensor_mul(out1[:], og[:], tnh[:])
                    dd = gp.tile([P, 8, BL], F32, tag="dd")
                    nc.vector.tensor_sub(dd[:], out1[:], pb[:])
                    hd = gp.tile([P, 8, BL], F32, tag="hd")
                    nc.vector.tensor_mul(hd[:], hg[:], dd[:])
                    out = sp.tile([P, 8, BL], F32, tag="out")
                    nc.vector.tensor_add(out[:], hd[:], pb[:])

                    st_new = sp.tile([P, 8, BL], F16, tag="st")
                    nc.vector.tensor_copy(st_new[:], out[:])

                    # pred = out.T @ WoT + b_out  -> [32, 151]
                    ps3 = psm.tile([BL, C], F32, tag="pred")
                    for k in range(8):
                        nc.tensor.matmul(ps3[:], out[:, k, :], WoT[:, k, :],
                                         start=(k == 0), stop=(k == 7))
                    pred = ap.tile([BL, C], F32, tag="predsb")
                    nc.vector.tensor_add(pred[:], ps3[:], bout[:])
                    nc.sync.dma_start(dists_d[t], pred[:])

                    # argmax over classes 1..150
                    mx = ap.tile([BL, 8], F32, tag="mx")
                    nc.vector.max(mx[:], pred[:, 1:C])
                    ix = ap.tile([BL, 8], U32, tag="ix")
                    nc.vector.max_index(ix[:], mx[:], pred[:, 1:C])
                    ixp = ap.tile([BL, 1], I32, tag="ixp")
                    nc.vector.tensor_scalar(ixp[:], ix[:, 0:1], 1, None, OP.add)
                    msk = ap.tile([BL, 1], I32, tag="msk")
                    nc.vector.tensor_scalar(msk[:], lab[:, t:t + 1], 0.0, None,
                                            OP.is_equal)
                    le = ap.tile([BL, 1], I32, tag="le")
                    nc.vector.tensor_copy(le[:], lab[:, t:t + 1])
                    nc.vector.copy_predicated(le[:], msk[:], ixp[:])
                    nc.vector.tensor_copy(comms_sb[:, t:t + 1], le[:])

                    # emb_next^T = obj_pad.T @ onehot(le+1)
                    lp1 = ap.tile([BL, 1], F32, tag="lp1")
                    nc.vector.tensor_scalar(lp1[:], le[:], 1, None, OP.add)
                    oh = ap.tile([BL, 256], F32, tag="oh")
                    nc.vector.tensor_scalar(oh[:], iota[:], lp1[:, 0:1], None,
                                            OP.is_equal)
                    ohT = ap.tile([P, 2, BL], F32, tag="ohT")
                    for hh in range(2):
                        ps4 = psm.tile([P, BL], F32, tag="me")
                        nc.tensor.transpose(ps4[:], oh[:, hh * P:(hh + 1) * P],
                                            ident[0:BL, 0:BL])
                        nc.vector.tensor_copy(ohT[:, hh, :], ps4[:])
                    ps5 = psm.tile([E, BL], F32, tag="me")
                    for cc in range(2):
                        nc.tensor.matmul(ps5[:], obj[:, cc, :], ohT[:, cc, :],
                                         start=(cc == 0), stop=(cc == 1))
                    emb_new = sp.tile([P, BL], F16, tag="emb")
                    nc.vector.tensor_copy(emb_new[0:E, :], ps5[:])

                    prev_state, prev_mem, prev_emb = st_new, mem_new, emb_new

                nc.sync.dma_start(comms_d[:, 0:t_steps], comms_sb[:, 0:t_steps])

    nc.compile()
    return nc


def prepare_inputs(seq, labels, W_in, b_in, W_state, b_state, W_out, b_out,
                   obj_embed, px_fp32=False):
    """Host-side prep: per-core shards + shared rearranged weights."""
    xdt = np.float32 if px_fp32 else np.float16
    seq = np.asarray(seq, np.float32)
    labels = np.asarray(labels, np.int32)
    W_in = np.asarray(W_in, np.float32)
    b_in = np.asarray(b_in, np.float32)
    W_state = np.asarray(W_state, np.float32)
    b_state = np.asarray(b_state, np.float32)
    W_out = np.asarray(W_out, np.float32)
    b_out = np.asarray(b_out, np.float32)
    obj_embed = np.asarray(obj_embed, np.float32)

    WxT = np.ascontiguousarray(W_in[:, :D_IN].T).astype(xdt)
    WeT = np.zeros((P, G6), np.float32)
    WeT[:E, :] = W_in[:, D_IN:].T
    WeT[:, G5:] *= 0.25  # bypass cols replicated x4 in psum groups
    WeT = WeT.astype(np.float16)
    WsT = np.ascontiguousarray(W_state.T).astype(np.float16)
    WoT = np.ascontiguousarray(W_out.T).astype(np.float32)
    objp = np.zeros((2 * P, E), np.float32)
    objp[:C + 2] = obj_embed
    bias_full = np.concatenate([b_in[:G5] + b_state, b_in[G5:]])
    biasf = np.ascontiguousarray(bias_full.reshape(48, P)).astype(np.float32)
    boutr = np.ascontiguousarray(np.broadcast_to(b_out, (BL, C))).astype(np.float32)
    iotar = np.ascontiguousarray(
        np.broadcast_to(np.arange(256, dtype=np.float32), (BL, 256)))
    ones4 = np.zeros((P, BL), np.float32)
    for p in range(P):
        ones4[p, p % BL] = 1.0
    ident = np.eye(P, dtype=np.float32)
    emb0T = np.zeros((P, BL), np.float32)
    emb0T[:E, :] = obj_embed[0][:, None]
    emb0T = emb0T.astype(np.float16)

    shared = dict(WxT=WxT, WeT=WeT, WsT=WsT, WoT=WoT, objp=objp, biasf=biasf,
                  boutr=boutr, iotar=iotar, ones4=ones4, ident=ident,
                  emb0T=emb0T)

    x3 = seq.reshape(T, B, D_IN)
    l2 = labels.reshape(T, B)
    in_maps = []
    for c in range(8):
        xs = np.ascontiguousarray(
            x3[:, c * BL:(c + 1) * BL, :].reshape(NTOK, D_IN).T).astype(xdt)
        labT = np.ascontiguousarray(l2[:, c * BL:(c + 1) * BL].T)
        m = dict(shared)
        m["xT"] = xs
        m["labT"] = labT
        in_maps.append(m)
    return in_maps


def assemble_outputs(results):
    dists = np.zeros((T, B, C), np.float32)
    comms = np.zeros((T, B), np.int32)
    for c, r in enumerate(results):
        dists[:, c * BL:(c + 1) * BL, :] = r["dists"]
        comms[:, c * BL:(c + 1) * BL] = r["comms"].T
    return dists.reshape(T * B, C), comms.reshape(T * B)


_CACHED = {}


def kernel(seq, labels, W_in, b_in, W_state, b_state, W_out, b_out, obj_embed):
    if "nc" not in _CACHED:
        _CACHED["nc"] = build_decoder()
    nc = _CACHED["nc"]
    in_maps = prepare_inputs(seq, labels, W_in, b_in, W_state, b_state,
                             W_out, b_out, obj_embed)
    res = run_bass_kernel_spmd(nc, in_maps, core_ids=list(range(8)))
    return assemble_outputs(res.results)


# revision 7
# speedup vs baseline: 20.1465x; 20.1465x over previous
"""DecoderRNN Trainium2 Bass kernel.

Data-parallel over batch: 8 cores x 32 batch rows each. Each core runs the
full T=128 recurrence for its shard; weights are replicated. The large
input projection (x @ W_x.T, 85% of FLOPs) is precomputed for all steps as
one big tiled matmul into DRAM; the recurrent loop then only does the
state/embedding projections per step.

Loop layout is feature-transposed ([feature-partitions, batch-free]) so the
elementwise gate math uses all 128 partitions. The state matmul uses a
two-stage trick: stage 1 packs the 8 K-chunks into the PE array 4-at-a-time
via tile_position col-groups (stationary = state slices, moving = W rows),
stage 2 reduces the 4 partition groups AND transposes via a matmul with a
block-ones matrix.

Precision: px/ps/pe matmuls in fp16 (inputs host-cast / device-rounded),
accumulation fp32; pred/argmax/embedding path exactly fp32.
"""

import numpy as np
import ml_dtypes

import concourse.bass as bass
import concourse.bacc as bacc
import concourse.mybir as mybir
from concourse.tile import TileContext
from concourse.bass_utils import run_bass_kernel_spmd

F32 = mybir.dt.float32
F16 = mybir.dt.float16
I32 = mybir.dt.int32
U32 = mybir.dt.uint32
AF = mybir.ActivationFunctionType
OP = mybir.AluOpType

T, B = 128, 256
D_IN, E, H, C = 2048, 100, 1024, 151
D = D_IN + E
BL = B // 8            # batch per core
G6, G5 = 6 * H, 5 * H
KD = D_IN // 128       # 16 contraction chunks for px
NTOK = T * BL          # 4096 tokens per core
P = 128


def build_decoder(t_steps=T, px_blocks=None, px_fp32=False):
    """Build the Bass kernel. t_steps/px_blocks < full for cheap sim tests."""
    xdt = F32 if px_fp32 else F16
    n_halves = 2 if px_fp32 else 1
    xcols = NTOK // n_halves
    blocks_per_half = xcols // 512
    if px_blocks is None:
        px_blocks = NTOK // 512

    nc = bacc.Bacc("TRN2", target_bir_lowering=False, num_devices=8)

    # ---- DRAM I/O ----
    xT_d = nc.dram_tensor("xT", [D_IN, NTOK], xdt, kind="ExternalInput")
    WxT_d = nc.dram_tensor("WxT", [D_IN, G6], xdt, kind="ExternalInput")
    WsT_d = nc.dram_tensor("WsT", [H, G5], F16, kind="ExternalInput")
    WeT_d = nc.dram_tensor("WeT", [P, G6], F16, kind="ExternalInput")
    WoT_d = nc.dram_tensor("WoT", [H, C], F32, kind="ExternalInput")
    obj_d = nc.dram_tensor("objp", [2 * P, E], F32, kind="ExternalInput")
    bias_d = nc.dram_tensor("biasf", [48, P], F32, kind="ExternalInput")
    bout_d = nc.dram_tensor("boutr", [BL, C], F32, kind="ExternalInput")
    iota_d = nc.dram_tensor("iotar", [BL, 256], F32, kind="ExternalInput")
    ones_d = nc.dram_tensor("ones4", [P, BL], F32, kind="ExternalInput")
    id_d = nc.dram_tensor("ident", [P, P], F32, kind="ExternalInput")
    lab_d = nc.dram_tensor("labT", [BL, T], I32, kind="ExternalInput")
    emb0_d = nc.dram_tensor("emb0T", [P, BL], F16, kind="ExternalInput")

    dists_d = nc.dram_tensor("dists", [T, BL, C], F32, kind="ExternalOutput")
    comms_d = nc.dram_tensor("comms", [BL, T], I32, kind="ExternalOutput")

    with TileContext(nc) as tc:
        with tc.tile_pool(name="dram", bufs=1, space="DRAM") as dpool, \
             tc.tile_pool(name="wsmall", bufs=1) as wp:
            # small persistent weights
            WeT = wp.tile([P, G6], F16)
            nc.sync.dma_start(WeT[:], WeT_d[:])
            WoT = wp.tile([P, 8, C], F32)
            nc.sync.dma_start(WoT[:], WoT_d[:].rearrange("(k p) c -> p k c", p=P))
            obj = wp.tile([P, 2, E], F32)
            nc.sync.dma_start(obj[:], obj_d[:].rearrange("(k p) e -> p k e", p=P))
            bias = wp.tile([P, 48], F32)
            nc.sync.dma_start(bias[:], bias_d[:].rearrange("m p -> p m"))
            bout = wp.tile([BL, C], F32)
            nc.sync.dma_start(bout[:], bout_d[:])
            iota = wp.tile([BL, 256], F32)
            nc.sync.dma_start(iota[:], iota_d[:])
            ones4 = wp.tile([P, BL], F32)
            nc.sync.dma_start(ones4[:], ones_d[:])
            ident = wp.tile([P, P], F32)
            nc.sync.dma_start(ident[:], id_d[:])
            lab = wp.tile([BL, T], I32)
            nc.sync.dma_start(lab[:], lab_d[:])
            emb0 = wp.tile([P, BL], F16)
            nc.sync.dma_start(emb0[:], emb0_d[:])

            px_dram = dpool.tile([48, NTOK // 512, P, 512], F32)

            # ================= PHASE A: px precompute =================
            with tc.tile_pool(name="pxc", bufs=1) as xp, \
                 tc.tile_pool(name="pxw", bufs=2) as wxp, \
                 tc.tile_pool(name="pxcp", bufs=4) as cpp, \
                 tc.tile_pool(name="pxps", bufs=4, space="PSUM") as pps:
                for h in range(n_halves):
                    xsb = xp.tile([P, KD, xcols], xdt, tag="xsb")
                    nc.sync.dma_start(
                        xsb[:],
                        xT_d[:, h * xcols:(h + 1) * xcols].rearrange(
                            "(k p) n -> p k n", p=P))
                    for m in range(48):
                        wsb = wxp.tile([P, KD, P], xdt, tag="wx")
                        nc.sync.dma_start(
                            wsb[:],
                            WxT_d[:, m * P:(m + 1) * P].rearrange(
                                "(k p) f -> p k f", p=P))
                        for nb in range(blocks_per_half):
                            gb = h * blocks_per_half + nb
                            if gb >= px_blocks:
                                continue
                            ps = pps.tile([P, 512], F32, tag="ps")
                            for k in range(KD):
                                nc.tensor.matmul(
                                    ps[:], wsb[:, k, :],
                                    xsb[:, k, nb * 512:(nb + 1) * 512],
                                    start=(k == 0), stop=(k == KD - 1))
                            cp = cpp.tile([P, 512], F32, tag="cp")
                            # copy + fold the (b_in + b_state) bias in
                            nc.scalar.activation(cp[:], ps[:], AF.Identity,
                                                 bias=bias[:, m:m + 1], scale=1.0)
                            nc.sync.dma_start(px_dram[m, gb], cp[:])

            # ================= PHASE B: recurrence =================
            with tc.tile_pool(name="wbig", bufs=1) as wb, \
                 tc.tile_pool(name="pxin", bufs=3) as pxp, \
                 tc.tile_pool(name="cpl", bufs=4) as cpl, \
                 tc.tile_pool(name="gat", bufs=2) as gp, \
                 tc.tile_pool(name="sta", bufs=2) as sp, \
                 tc.tile_pool(name="arg", bufs=2) as ap, \
                 tc.tile_pool(name="ps1", bufs=3, space="PSUM") as ps1p, \
                 tc.tile_pool(name="ps2", bufs=1, space="PSUM") as ps2p, \
                 tc.tile_pool(name="psm", bufs=1, space="PSUM") as psm:

                WsT = wb.tile([P, 8, G5], F16)
                nc.sync.dma_start(WsT[:], WsT_d[:].rearrange("(k p) g -> p k g", p=P))
                comms_sb = wb.tile([BL, T], I32)

                prev_state = wb.tile([P, 8, BL], F16)
                nc.vector.memset(prev_state[:], 0)
                prev_mem = wb.tile([P, 8, BL], F32)
                nc.vector.memset(prev_mem[:], 0)
                prev_emb = emb0

                FUNC = [AF.Sigmoid, AF.Sigmoid, AF.Tanh, AF.Sigmoid, AF.Sigmoid]

                for t in range(t_steps):
                    # px for this step: [128, 48, 32]
                    pxt = pxp.tile([P, 48, BL], F32, tag="px")
                    nc.sync.dma_start(
                        pxt[:],
                        px_dram[:, t // 16, :,
                                (t % 16) * BL:(t % 16 + 1) * BL].rearrange(
                                    "m p j -> p m j"))

                    # stage 1: psum1[32g+n, f] = partial ps sums (+ pe in grp0)
                    cps = []
                    for c in range(12):
                        ps1 = ps1p.tile([P, 512], F32, tag="s1")
                        csl = slice(c * 512, (c + 1) * 512)
                        if c < 10:
                            for r in range(2):
                                for g in range(4):
                                    k = r * 4 + g
                                    nc.tensor.matmul(
                                        ps1[32 * g:32 * (g + 1), :],
                                        prev_state[:, k, :], WsT[:, k, csl],
                                        start=(r == 0), stop=(r == 1 and g == 3),
                                        tile_position=(0, 32 * g),
                                        skip_group_check=True)
                            nc.tensor.matmul(
                                ps1[0:32, :], prev_emb[0:E, :], WeT[0:E, csl],
                                start=False, stop=True, tile_position=(0, 0),
                                skip_group_check=True)
                        else:
                            # bypass cols: pe only, replicated in all 4 groups
                            # (WeT bypass cols pre-scaled by 0.25 on host)
                            for g in range(4):
                                nc.tensor.matmul(
                                    ps1[32 * g:32 * (g + 1), :],
                                    prev_emb[0:E, :], WeT[0:E, csl],
                                    start=True, stop=True,
                                    tile_position=(0, 32 * g),
                                    skip_group_check=True)
                        cp = cpl.tile([P, 512], F32, tag="cpl")
                        if c % 2 == 0:
                            nc.scalar.copy(cp[:], ps1[:])
                        else:
                            nc.vector.tensor_copy(cp[:], ps1[:])
                        cps.append(cp)

                    # stage 2: reduce groups + transpose -> ps2[p, fc, b]
                    ps2 = ps2p.tile([P, 48, BL], F32, tag="s2")
                    for c in range(12):
                        for s in range(4):
                            fc = c * 4 + s
                            nc.tensor.matmul(
                                ps2[:, fc, :], cps[c][:, s * P:(s + 1) * P],
                                ones4[:], start=True, stop=True)

                    # gates
                    gates = []
                    for g in range(5):
                        pre = gp.tile([P, 8, BL], F32, tag=f"pre{g}")
                        nc.vector.tensor_add(pre[:], ps2[:, 8 * g:8 * (g + 1), :],
                                             pxt[:, 8 * g:8 * (g + 1), :])
                        gt = gp.tile([P, 8, BL], F32, tag=f"g{g}")
                        nc.scalar.activation(gt[:], pre[:], FUNC[g])
                        gates.append(gt)
                    ig, fg, mi, og, hg = gates
                    pb = gp.tile([P, 8, BL], F32, tag="pb")
                    nc.vector.tensor_add(pb[:], ps2[:, 40:48, :], pxt[:, 40:48, :])

                    t1 = gp.tile([P, 8, BL], F32, tag="t1")
                    nc.vector.tensor_mul(t1[:], ig[:], mi[:])
                    t2 = gp.tile([P, 8, BL], F32, tag="t2")
                    nc.vector.tensor_mul(t2[:], fg[:], prev_mem[:])
                    mem_new = sp.tile([P, 8, BL], F32, tag="mem")
                    nc.vector.tensor_add(mem_new[:], t1[:], t2[:])
                    tnh = gp.tile([P, 8, BL], F32, tag="tnh")
                    nc.scalar.activation(tnh[:], mem_new[:], AF.Tanh)
                    out1 = gp.tile([P, 8, BL], F32, tag="out1")
                    nc.vector.tensor_mul(out1[:], og[:], tnh[:])
                    dd = gp.tile([P, 8, BL], F32, tag="dd")
                    nc.vector.tensor_sub(dd[:], out1[:], pb[:])
                    hd = gp.tile([P, 8, BL], F32, tag="hd")
                    nc.vector.tensor_mul(hd[:], hg[:], dd[:])
                    out = sp.tile([P, 8, BL], F32, tag="out")
                    nc.vector.tensor_add(out[:], hd[:], pb[:])

                    st_new = sp.tile([P, 8, BL], F16, tag="st")
                    nc.vector.tensor_copy(st_new[:], out[:])

                    # pred = out.T @ WoT + b_out  -> [32, 151]
                    ps3 = psm.tile([BL, C], F32, tag="pred")
                    for k in range(8):
                        nc.tensor.matmul(ps3[:], out[:, k, :], WoT[:, k, :],
                                         start=(k == 0), stop=(k == 7))
                    pred = ap.tile([BL, C], F32, tag="predsb")
                    nc.vector.tensor_add(pred[:], ps3[:], bout[:])
                    nc.sync.dma_start(dists_d[t], pred[:])

                    # argmax over classes 1..150
                    mx = ap.tile([BL, 8], F32, tag="mx")
                    nc.vector.max(mx[:], pred[:, 1:C])
                    ix = ap.tile([BL, 8], U32, tag="ix")
                    nc.vector.max_index(ix[:], mx[:], pred[:, 1:C])
                    ixp = ap.tile([BL, 1], I32, tag="ixp")
                    nc.vector.tensor_scalar(ixp[:], ix[:, 0:1], 1, None, OP.add)
                    msk = ap.tile([BL, 1], I32, tag="msk")
                    nc.vector.tensor_scalar(msk[:], lab[:, t:t + 1], 0.0, None,
                                            OP.is_equal)
                    le = ap.tile([BL, 1], I32, tag="le")
                    nc.vector.tensor_copy(le[:], lab[:, t:t + 1])
                    nc.vector.copy_predicated(le[:], msk[:], ixp[:])
                    nc.vector.tensor_copy(comms_sb[:, t:t + 1], le[:])

                    # emb_next^T = obj_pad.T @ onehot(le+1)
                    lp1 = ap.tile([BL, 1], F32, tag="lp1")
                    nc.vector.tensor_scalar(lp1[:], le[:], 1, None, OP.add)
                    oh = ap.tile([BL, 256], F32, tag="oh")
                    nc.vector.tensor_scalar(oh[:], iota[:], lp1[:, 0:1], None,
                                            OP.is_equal)
                    ohT = ap.tile([P, 2, BL], F32, tag="ohT")
                    for hh in range(2):
                        ps4 = psm.tile([P, BL], F32, tag="me")
                        nc.tensor.transpose(ps4[:], oh[:, hh * P:(hh + 1) * P],
                                            ident[0:BL, 0:BL])
                        nc.vector.tensor_copy(ohT[:, hh, :], ps4[:])
                    ps5 = psm.tile([E, BL], F32, tag="me")
                    for cc in range(2):
                        nc.tensor.matmul(ps5[:], obj[:, cc, :], ohT[:, cc, :],
                                         start=(cc == 0), stop=(cc == 1))
                    emb_new = sp.tile([P, BL], F16, tag="emb")
                    nc.vector.tensor_copy(emb_new[0:E, :], ps5[:])

                    prev_state, prev_mem, prev_emb = st_new, mem_new, emb_new

                nc.sync.dma_start(comms_d[:, 0:t_steps], comms_sb[:, 0:t_steps])

    nc.compile()
    return nc


def prepare_inputs(seq, labels, W_in, b_in, W_state, b_state, W_out, b_out,
                   obj_embed, px_fp32=False):
    """Host-side prep: per-core shards + shared rearranged weights."""
    xdt = np.float32 if px_fp32 else np.float16
    seq = np.asarray(seq, np.float32)
    labels = np.asarray(labels, np.int32)
    W_in = np.asarray(W_in, np.float32)
    b_in = np.asarray(b_in, np.float32)
    W_state = np.asarray(W_state, np.float32)
    b_state = np.asarray(b_state, np.float32)
    W_out = np.asarray(W_out, np.float32)
    b_out = np.asarray(b_out, np.float32)
    obj_embed = np.asarray(obj_embed, np.float32)

    WxT = np.ascontiguousarray(W_in[:, :D_IN].T).astype(xdt)
    WeT = np.zeros((P, G6), np.float32)
    WeT[:E, :] = W_in[:, D_IN:].T
    WeT[:, G5:] *= 0.25  # bypass cols replicated x4 in psum groups
    WeT = WeT.astype(np.float16)
    WsT = np.ascontiguousarray(W_state.T).astype(np.float16)
    WoT = np.ascontiguousarray(W_out.T).astype(np.float32)
    objp = np.zeros((2 * P, E), np.float32)
    objp[:C + 2] = obj_embed
    bias_full = np.concatenate([b_in[:G5] + b_state, b_in[G5:]])
    biasf = np.ascontiguousarray(bias_full.reshape(48, P)).astype(np.float32)
    boutr = np.ascontiguousarray(np.broadcast_to(b_out, (BL, C))).astype(np.float32)
    iotar = np.ascontiguousarray(
        np.broadcast_to(np.arange(256, dtype=np.float32), (BL, 256)))
    ones4 = np.zeros((P, BL), np.float32)
    for p in range(P):
        ones4[p, p % BL] = 1.0
    ident = np.eye(P, dtype=np.float32)
    emb0T = np.zeros((P, BL), np.float32)
    emb0T[:E, :] = obj_embed[0][:, None]
    emb0T = emb0T.astype(np.float16)

    shared = dict(WxT=WxT, WeT=WeT, WsT=WsT, WoT=WoT, objp=objp, biasf=biasf,
                  boutr=boutr, iotar=iotar, ones4=ones4, ident=ident,
                  emb0T=emb0T)

    x3 = seq.reshape(T, B, D_IN)
    l2 = labels.reshape(T, B)
    in_maps = []
    for c in range(8):
        xs = np.ascontiguousarray(
            x3[:, c * BL:(c + 1) * BL, :].reshape(NTOK, D_IN).T).astype(xdt)
        labT = np.ascontiguousarray(l2[:, c * BL:(c + 1) * BL].T)
        m = dict(shared)
        m["xT"] = xs
        m["labT"] = labT
        in_maps.append(m)
    return in_maps


def assemble_outputs(results):
    dists = np.zeros((T, B, C), np.float32)
    comms = np.zeros((T, B), np.int32)
    for c, r in enumerate(results):
        dists[:, c * BL:(c + 1) * BL, :] = r["dists"]
        comms[:, c * BL:(c + 1) * BL] = r["comms"].T
    return dists.reshape(T * B, C), comms.reshape(T * B)


_CACHED = {}


def kernel(seq, labels, W_in, b_in, W_state, b_state, W_out, b_out, obj_embed):
    if "nc" not in _CACHED:
        _CACHED["nc"] = build_decoder()
    nc = _CACHED["nc"]
    in_maps = prepare_inputs(seq, labels, W_in, b_in, W_state, b_state,
                             W_out, b_out, obj_embed)
    res = run_bass_kernel_spmd(nc, in_maps, core_ids=list(range(8)))
    return assemble_outputs(res.results)
